# revision 4
# baseline (speedup 1.0000x reference)
"""Trainium2 Bass kernel for nn_EnsembleModel (ensemble MLP, M=8 models).

Sharding: one ensemble member per NeuronCore (8 models / 8 cores). Each core
runs the full batch through its model's 3-layer MLP + 4 output heads.

Layout: features on partitions, batch on the free dim ("transposed"
activations), so every layer is out[h_out, b] = W_chunk.T @ h_prev[h_in, b]
with no transposes anywhere. The input x.T and all weight reshapes are done
host-side in numpy; outputs come back as [130, B] per core and are
untransposed host-side.

Matmuls run as float32r (full fp32 storage, full-rate PE mode at free dim
>= 256). tanh + per-feature bias are fused into one ScalarE activation per
128-row chunk. The soft log-var clamp
    lv = -10 + softplus(10.5 - softplus(0.5 - lv))
is computed exactly as b - a with
    u = exp(0.5 - lv), a = ln(1 + u), b = ln(e^-10 * u + (e^-10 + e^0.5))
which needs only Exp and Ln (one ACT table-set switch per tile pair).
"""

import math

import numpy as np

M, B, OBS, ACT, H = 8, 4096, 64, 32, 1024
IN = OBS + ACT  # 96
P = 128
KC = H // P  # 8 k-chunks per 1024-dim contraction
NH = 2 * OBS + 2  # 130 head output columns: [mu_o(64), mu_r(1), v_o(64), v_r(1)]
B_T = 512
N_BT = B // B_T
MAX_LV, MIN_LV = 0.5, -10.0

_CLAMP_SCALE = float(np.exp(MIN_LV))  # e^-10
_CLAMP_BIAS = float(np.exp(MIN_LV) + np.exp(MAX_LV))  # e^-10 + e^0.5

_PROGRAM = None


def _build_program():
    import concourse.mybir as mybir
    from concourse import bacc
    from concourse.bass import ds, ts
    from concourse.tile import TileContext

    f32 = mybir.dt.float32
    f32r = mybir.dt.float32r
    Act = mybir.ActivationFunctionType

    nc = bacc.Bacc("TRN2", target_bir_lowering=False)

    xT = nc.dram_tensor("xT", [P, B], f32r, kind="ExternalInput")
    w0 = nc.dram_tensor("w0", [P, H], f32r, kind="ExternalInput")
    w1 = nc.dram_tensor("w1", [P, KC, H], f32r, kind="ExternalInput")
    w2 = nc.dram_tensor("w2", [P, KC, H], f32r, kind="ExternalInput")
    wh = nc.dram_tensor("wh", [P, KC, NH], f32r, kind="ExternalInput")
    b0 = nc.dram_tensor("b0", [P, KC], f32, kind="ExternalInput")
    b1 = nc.dram_tensor("b1", [P, KC], f32, kind="ExternalInput")
    b2 = nc.dram_tensor("b2", [P, KC], f32, kind="ExternalInput")
    bh = nc.dram_tensor("bh", [P, 3], f32, kind="ExternalInput")
    out = nc.dram_tensor("out", [NH, B], f32, kind="ExternalOutput")

    def r(ap):
        return ap  # tiles feeding matmuls are already float32r

    with TileContext(nc) as tc:
        with (
            tc.tile_pool(name="consts", bufs=1) as consts,
            tc.tile_pool(name="h0p", bufs=2) as h0p,
            tc.tile_pool(name="h1p", bufs=2) as h1p,
            tc.tile_pool(name="h2p", bufs=1) as h2p,
            tc.tile_pool(name="epi", bufs=2) as epi,
            tc.tile_pool(name="psum", bufs=8, space="PSUM") as psum_pool,
        ):
            xT_sb = consts.tile([P, B], f32r, tag="xT")
            w0_sb = consts.tile([P, H], f32r, tag="w0")
            w1_sb = consts.tile([P, KC, H], f32r, tag="w1")
            w2_sb = consts.tile([P, KC, H], f32r, tag="w2")
            wh_sb = consts.tile([P, KC, NH], f32r, tag="wh")
            b0_sb = consts.tile([P, KC], f32, tag="b0")
            b1_sb = consts.tile([P, KC], f32, tag="b1")
            b2_sb = consts.tile([P, KC], f32, tag="b2")
            bh_sb = consts.tile([P, 3], f32, tag="bh")

            # Inputs needed first go first so layer-0/1 matmuls aren't gated
            # on the tail of the weight transfer.
            nc.sync.dma_start(xT_sb[:], xT[:])
            nc.sync.dma_start(w0_sb[:], w0[:])
            nc.sync.dma_start(b0_sb[:], b0[:])
            for k in range(KC):
                nc.sync.dma_start(w1_sb[:, k], w1[:, k])
            nc.sync.dma_start(b1_sb[:], b1[:])
            for k in range(KC):
                nc.sync.dma_start(w2_sb[:, k], w2[:, k])
            nc.sync.dma_start(b2_sb[:], b2[:])
            nc.sync.dma_start(wh_sb[:], wh[:])
            nc.sync.dma_start(bh_sb[:], bh[:])

            for j in range(N_BT):
                js = ds(j * B_T, B_T)

                # Layer 0: [96->128 padded, B_T] -> h0 [1024, B_T]
                h0 = h0p.tile([P, KC, B_T], f32r)
                for c in range(KC):
                    ps = psum_pool.tile([P, B_T], f32, tag="ps")
                    nc.tensor.matmul(
                        ps[:], r(w0_sb[:, ts(c, P)]), r(xT_sb[:, js]),
                        start=True, stop=True,
                    )
                    nc.scalar.activation(
                        h0[:, c], ps[:], Act.Tanh, bias=b0_sb[:, c : c + 1]
                    )

                # Layers 1 and 2: 1024 -> 1024, k-accumulated in PSUM
                h1 = h1p.tile([P, KC, B_T], f32r)
                for c in range(KC):
                    ps = psum_pool.tile([P, B_T], f32, tag="ps")
                    for k in range(KC):
                        nc.tensor.matmul(
                            ps[:], r(w1_sb[:, k, ts(c, P)]), r(h0[:, k]),
                            start=(k == 0), stop=(k == KC - 1),
                        )
                    nc.scalar.activation(
                        h1[:, c], ps[:], Act.Tanh, bias=b1_sb[:, c : c + 1]
                    )

                h2 = h2p.tile([P, KC, B_T], f32r)
                for c in range(KC):
                    ps = psum_pool.tile([P, B_T], f32, tag="ps")
                    for k in range(KC):
                        nc.tensor.matmul(
                            ps[:], r(w2_sb[:, k, ts(c, P)]), r(h1[:, k]),
                            start=(k == 0), stop=(k == KC - 1),
                        )
                    nc.scalar.activation(
                        h2[:, c], ps[:], Act.Tanh, bias=b2_sb[:, c : c + 1]
                    )

                # Heads: two 65-row groups ([mu_o, mu_r] and [v_o, v_r])
                ps_mu = psum_pool.tile([P, B_T], f32, tag="ps")
                ps_lv = psum_pool.tile([P, B_T], f32, tag="ps")
                for k in range(KC):
                    nc.tensor.matmul(
                        ps_mu[0 : OBS + 1], r(wh_sb[:, k, 0 : OBS + 1]), r(h2[:, k]),
                        start=(k == 0), stop=(k == KC - 1),
                    )
                for k in range(KC):
                    nc.tensor.matmul(
                        ps_lv[0 : OBS + 1], r(wh_sb[:, k, OBS + 1 : NH]), r(h2[:, k]),
                        start=(k == 0), stop=(k == KC - 1),
                    )

                mu_sb = epi.tile([OBS + 1, B_T], f32, tag="mu")
                nc.scalar.activation(
                    mu_sb[:], ps_mu[0 : OBS + 1], Act.Identity,
                    bias=bh_sb[0 : OBS + 1, 0:1],
                )
                nc.sync.dma_start(out[0 : OBS + 1, js], mu_sb[:])

                # u = exp(0.5 - (pre + bias_v)); bias column already holds 0.5 - bias_v
                u_sb = epi.tile([OBS + 1, B_T], f32, tag="u")
                nc.scalar.activation(
                    u_sb[:], ps_lv[0 : OBS + 1], Act.Exp,
                    bias=bh_sb[0 : OBS + 1, 1:2], scale=-1.0,
                )
                a_sb = epi.tile([OBS + 1, B_T], f32, tag="a")
                nc.scalar.activation(a_sb[:], u_sb[:], Act.Ln, bias=1.0)
                b_sb = epi.tile([OBS + 1, B_T], f32, tag="b")
                nc.scalar.activation(
                    b_sb[:], u_sb[:], Act.Ln,
                    bias=bh_sb[0 : OBS + 1, 2:3], scale=_CLAMP_SCALE,
                )
                nc.vector.tensor_sub(b_sb[:], b_sb[:], a_sb[:])
                nc.sync.dma_start(out[OBS + 1 : NH, js], b_sb[:])

    nc.finalize()
    return nc


def _get_program():
    global _PROGRAM
    if _PROGRAM is None:
        _PROGRAM = _build_program()
    return _PROGRAM


def _make_in_maps(inputs):
    obs = np.asarray(inputs["observation"], np.float32)
    act = np.asarray(inputs["action"], np.float32)
    x = np.concatenate([obs, act], axis=1)  # [B, IN]
    xT = np.zeros((P, B), np.float32)
    xT[:IN] = x.T

    W0, b0 = np.asarray(inputs["W0"], np.float32), np.asarray(inputs["b0"], np.float32)
    W1, b1 = np.asarray(inputs["W1"], np.float32), np.asarray(inputs["b1"], np.float32)
    W2, b2 = np.asarray(inputs["W2"], np.float32), np.asarray(inputs["b2"], np.float32)
    Wmu_o, bmu_o = np.asarray(inputs["Wmu_o"], np.float32), np.asarray(inputs["bmu_o"], np.float32)
    Wmu_r, bmu_r = np.asarray(inputs["Wmu_r"], np.float32), np.asarray(inputs["bmu_r"], np.float32)
    Wv_o, bv_o = np.asarray(inputs["Wv_o"], np.float32), np.asarray(inputs["bv_o"], np.float32)
    Wv_r, bv_r = np.asarray(inputs["Wv_r"], np.float32), np.asarray(inputs["bv_r"], np.float32)

    def kchunk(w, ncols):
        # [H, ncols] -> [128, KC, ncols] with row index = ko*128 + ki
        return np.ascontiguousarray(w.reshape(KC, P, ncols).transpose(1, 0, 2))

    in_maps = []
    for m in range(M):
        w0p = np.zeros((P, H), np.float32)
        w0p[:IN] = W0[m]
        whm = np.concatenate([Wmu_o[m], Wmu_r[m], Wv_o[m], Wv_r[m]], axis=1)  # [H, NH]
        bhm = np.zeros((P, 3), np.float32)
        bhm[0:OBS, 0] = bmu_o[m]
        bhm[OBS, 0] = bmu_r[m, 0]
        bhm[0:OBS, 1] = MAX_LV - bv_o[m]
        bhm[OBS, 1] = MAX_LV - bv_r[m, 0]
        bhm[:, 2] = _CLAMP_BIAS
        in_maps.append(
            {
                "xT": xT,
                "w0": w0p,
                "w1": kchunk(W1[m], H),
                "w2": kchunk(W2[m], H),
                "wh": kchunk(whm, NH),
                "b0": np.ascontiguousarray(b0[m].reshape(KC, P).T),
                "b1": np.ascontiguousarray(b1[m].reshape(KC, P).T),
                "b2": np.ascontiguousarray(b2[m].reshape(KC, P).T),
                "bh": bhm,
            }
        )
    return in_maps


def _unshard(results):
    outs = [np.asarray(res["out"], np.float32) for res in results]  # [130, B] each
    mu_o = np.stack([np.ascontiguousarray(o[0:OBS].T) for o in outs])
    mu_r = np.stack([np.ascontiguousarray(o[OBS : OBS + 1].T) for o in outs])
    lv_o = np.stack([np.ascontiguousarray(o[OBS + 1 : 2 * OBS + 1].T) for o in outs])
    lv_r = np.stack([np.ascontiguousarray(o[2 * OBS + 1 : NH].T) for o in outs])
    return mu_o, lv_o, mu_r, lv_r


def run(inputs, trace=False, **spmd_kwargs):
    """Run the SPMD kernel; returns ((mu_o, lv_o, mu_r, lv_r), BassKernelResults)."""
    from concourse.bass_utils import run_bass_kernel_spmd

    nc = _get_program()
    in_maps = _make_in_maps(inputs)
    res = run_bass_kernel_spmd(
        nc, in_maps, core_ids=list(range(M)), trace=trace, **spmd_kwargs
    )
    return _unshard(res.results), res


def kernel(**inputs):
    outputs, _ = run(inputs)
    return outputs


# revision 5
# speedup vs baseline: 4.3547x; 4.3547x over previous
"""Trainium2 Bass kernel for nn_EnsembleModel (ensemble MLP, M=8 models).

Sharding: one ensemble member per NeuronCore (8 models / 8 cores). Each core
runs the full batch through its model's 3-layer MLP + 4 output heads.

Layout: features on partitions, batch on the free dim ("transposed"
activations), so every layer is out[h_out, b] = W_chunk.T @ h_prev[h_in, b]
with no transposes anywhere. The input x.T and all weight reshapes are done
host-side in numpy; outputs come back as [130, B] per core and are
untransposed host-side.

Matmuls run as float32r (full fp32 storage, full-rate PE mode at free dim
>= 256). tanh + per-feature bias are fused into one ScalarE activation per
128-row chunk. The soft log-var clamp
    lv = -10 + softplus(10.5 - softplus(0.5 - lv))
is computed exactly as b - a with
    u = exp(0.5 - lv), a = ln(1 + u), b = ln(e^-10 * u + (e^-10 + e^0.5))
which needs only Exp and Ln (one ACT table-set switch per tile pair).
"""

import math

import numpy as np

M, B, OBS, ACT, H = 8, 4096, 64, 32, 1024
IN = OBS + ACT  # 96
P = 128
KC = H // P  # 8 k-chunks per 1024-dim contraction
NH = 2 * OBS + 2  # 130 head output columns: [mu_o(64), mu_r(1), v_o(64), v_r(1)]
B_T = 512
N_BT = B // B_T
MAX_LV, MIN_LV = 0.5, -10.0

_CLAMP_SCALE = float(np.exp(MIN_LV))  # e^-10
_CLAMP_BIAS = float(np.exp(MIN_LV) + np.exp(MAX_LV))  # e^-10 + e^0.5

_PROGRAM = None


def _build_program(repeat=1):
    import concourse.mybir as mybir
    from concourse import bacc
    from concourse.bass import ds, ts
    from concourse.tile import TileContext

    f32 = mybir.dt.float32
    f32r = mybir.dt.float32r
    Act = mybir.ActivationFunctionType

    nc = bacc.Bacc("TRN2", target_bir_lowering=False)

    xT = nc.dram_tensor("xT", [P, B], f32r, kind="ExternalInput")
    w0 = nc.dram_tensor("w0", [P, H], f32r, kind="ExternalInput")
    w1 = nc.dram_tensor("w1", [P, KC, H], f32r, kind="ExternalInput")
    w2 = nc.dram_tensor("w2", [P, KC, H], f32r, kind="ExternalInput")
    wh = nc.dram_tensor("wh", [P, KC, NH], f32r, kind="ExternalInput")
    b0 = nc.dram_tensor("b0", [P, KC], f32, kind="ExternalInput")
    b1 = nc.dram_tensor("b1", [P, KC], f32, kind="ExternalInput")
    b2 = nc.dram_tensor("b2", [P, KC], f32, kind="ExternalInput")
    bh = nc.dram_tensor("bh", [P, 3], f32, kind="ExternalInput")
    out = nc.dram_tensor("out", [NH, B], f32, kind="ExternalOutput")

    def r(ap):
        return ap  # tiles feeding matmuls are already float32r

    with TileContext(nc) as tc:
        with (
            tc.tile_pool(name="consts", bufs=1) as consts,
            tc.tile_pool(name="h0p", bufs=2) as h0p,
            tc.tile_pool(name="h1p", bufs=2) as h1p,
            tc.tile_pool(name="h2p", bufs=1) as h2p,
            tc.tile_pool(name="epi", bufs=2) as epi,
            tc.tile_pool(name="psum", bufs=8, space="PSUM") as psum_pool,
        ):
            xT_sb = consts.tile([P, B], f32r, tag="xT")
            w0_sb = consts.tile([P, H], f32r, tag="w0")
            w1_sb = consts.tile([P, KC, H], f32r, tag="w1")
            w2_sb = consts.tile([P, KC, H], f32r, tag="w2")
            wh_sb = consts.tile([P, KC, NH], f32r, tag="wh")
            b0_sb = consts.tile([P, KC], f32, tag="b0")
            b1_sb = consts.tile([P, KC], f32, tag="b1")
            b2_sb = consts.tile([P, KC], f32, tag="b2")
            bh_sb = consts.tile([P, 3], f32, tag="bh")

            # Inputs needed first go first so layer-0/1 matmuls aren't gated
            # on the tail of the weight transfer.
            nc.sync.dma_start(xT_sb[:], xT[:])
            nc.sync.dma_start(w0_sb[:], w0[:])
            nc.sync.dma_start(b0_sb[:], b0[:])
            for k in range(KC):
                nc.sync.dma_start(w1_sb[:, k], w1[:, k])
            nc.sync.dma_start(b1_sb[:], b1[:])
            for k in range(KC):
                nc.sync.dma_start(w2_sb[:, k], w2[:, k])
            nc.sync.dma_start(b2_sb[:], b2[:])
            nc.sync.dma_start(wh_sb[:], wh[:])
            nc.sync.dma_start(bh_sb[:], bh[:])

            for j in range(N_BT * repeat):
                j = j % N_BT
                js = ds(j * B_T, B_T)

                # Layer 0: [96->128 padded, B_T] -> h0 [1024, B_T]
                h0 = h0p.tile([P, KC, B_T], f32r)
                for c in range(KC):
                    ps = psum_pool.tile([P, B_T], f32, tag="ps")
                    nc.tensor.matmul(
                        ps[:], r(w0_sb[:, ts(c, P)]), r(xT_sb[:, js]),
                        start=True, stop=True,
                    )
                    nc.scalar.activation(
                        h0[:, c], ps[:], Act.Tanh, bias=b0_sb[:, c : c + 1]
                    )

                # Layers 1 and 2: 1024 -> 1024, k-accumulated in PSUM
                h1 = h1p.tile([P, KC, B_T], f32r)
                for c in range(KC):
                    ps = psum_pool.tile([P, B_T], f32, tag="ps")
                    for k in range(KC):
                        nc.tensor.matmul(
                            ps[:], r(w1_sb[:, k, ts(c, P)]), r(h0[:, k]),
                            start=(k == 0), stop=(k == KC - 1),
                        )
                    nc.scalar.activation(
                        h1[:, c], ps[:], Act.Tanh, bias=b1_sb[:, c : c + 1]
                    )

                h2 = h2p.tile([P, KC, B_T], f32r)
                for c in range(KC):
                    ps = psum_pool.tile([P, B_T], f32, tag="ps")
                    for k in range(KC):
                        nc.tensor.matmul(
                            ps[:], r(w2_sb[:, k, ts(c, P)]), r(h1[:, k]),
                            start=(k == 0), stop=(k == KC - 1),
                        )
                    nc.scalar.activation(
                        h2[:, c], ps[:], Act.Tanh, bias=b2_sb[:, c : c + 1]
                    )

                # Heads: two 65-row groups ([mu_o, mu_r] and [v_o, v_r])
                ps_mu = psum_pool.tile([P, B_T], f32, tag="ps")
                ps_lv = psum_pool.tile([P, B_T], f32, tag="ps")
                for k in range(KC):
                    nc.tensor.matmul(
                        ps_mu[0 : OBS + 1], r(wh_sb[:, k, 0 : OBS + 1]), r(h2[:, k]),
                        start=(k == 0), stop=(k == KC - 1),
                    )
                for k in range(KC):
                    nc.tensor.matmul(
                        ps_lv[0 : OBS + 1], r(wh_sb[:, k, OBS + 1 : NH]), r(h2[:, k]),
                        start=(k == 0), stop=(k == KC - 1),
                    )

                mu_sb = epi.tile([OBS + 1, B_T], f32, tag="mu")
                nc.scalar.activation(
                    mu_sb[:], ps_mu[0 : OBS + 1], Act.Identity,
                    bias=bh_sb[0 : OBS + 1, 0:1],
                )
                nc.sync.dma_start(out[0 : OBS + 1, js], mu_sb[:])

                # u = exp(0.5 - (pre + bias_v)); bias column already holds 0.5 - bias_v
                u_sb = epi.tile([OBS + 1, B_T], f32, tag="u")
                nc.scalar.activation(
                    u_sb[:], ps_lv[0 : OBS + 1], Act.Exp,
                    bias=bh_sb[0 : OBS + 1, 1:2], scale=-1.0,
                )
                a_sb = epi.tile([OBS + 1, B_T], f32, tag="a")
                nc.scalar.activation(a_sb[:], u_sb[:], Act.Ln, bias=1.0)
                b_sb = epi.tile([OBS + 1, B_T], f32, tag="b")
                nc.scalar.activation(
                    b_sb[:], u_sb[:], Act.Ln,
                    bias=bh_sb[0 : OBS + 1, 2:3], scale=_CLAMP_SCALE,
                )
                nc.vector.tensor_sub(b_sb[:], b_sb[:], a_sb[:])
                nc.sync.dma_start(out[OBS + 1 : NH, js], b_sb[:])

    nc.finalize()
    return nc


def _get_program():
    global _PROGRAM
    if _PROGRAM is None:
        _PROGRAM = _build_program()
    return _PROGRAM


def _get_repeat_program(repeat):
    return _build_program(repeat=repeat)


def _make_in_maps(inputs):
    obs = np.asarray(inputs["observation"], np.float32)
    act = np.asarray(inputs["action"], np.float32)
    x = np.concatenate([obs, act], axis=1)  # [B, IN]
    xT = np.zeros((P, B), np.float32)
    xT[:IN] = x.T

    W0, b0 = np.asarray(inputs["W0"], np.float32), np.asarray(inputs["b0"], np.float32)
    W1, b1 = np.asarray(inputs["W1"], np.float32), np.asarray(inputs["b1"], np.float32)
    W2, b2 = np.asarray(inputs["W2"], np.float32), np.asarray(inputs["b2"], np.float32)
    Wmu_o, bmu_o = np.asarray(inputs["Wmu_o"], np.float32), np.asarray(inputs["bmu_o"], np.float32)
    Wmu_r, bmu_r = np.asarray(inputs["Wmu_r"], np.float32), np.asarray(inputs["bmu_r"], np.float32)
    Wv_o, bv_o = np.asarray(inputs["Wv_o"], np.float32), np.asarray(inputs["bv_o"], np.float32)
    Wv_r, bv_r = np.asarray(inputs["Wv_r"], np.float32), np.asarray(inputs["bv_r"], np.float32)

    def kchunk(w, ncols):
        # [H, ncols] -> [128, KC, ncols] with row index = ko*128 + ki
        return np.ascontiguousarray(w.reshape(KC, P, ncols).transpose(1, 0, 2))

    in_maps = []
    for m in range(M):
        w0p = np.zeros((P, H), np.float32)
        w0p[:IN] = W0[m]
        whm = np.concatenate([Wmu_o[m], Wmu_r[m], Wv_o[m], Wv_r[m]], axis=1)  # [H, NH]
        bhm = np.zeros((P, 3), np.float32)
        bhm[0:OBS, 0] = bmu_o[m]
        bhm[OBS, 0] = bmu_r[m, 0]
        bhm[0:OBS, 1] = MAX_LV - bv_o[m]
        bhm[OBS, 1] = MAX_LV - bv_r[m, 0]
        bhm[:, 2] = _CLAMP_BIAS
        in_maps.append(
            {
                "xT": xT,
                "w0": w0p,
                "w1": kchunk(W1[m], H),
                "w2": kchunk(W2[m], H),
                "wh": kchunk(whm, NH),
                "b0": np.ascontiguousarray(b0[m].reshape(KC, P).T),
                "b1": np.ascontiguousarray(b1[m].reshape(KC, P).T),
                "b2": np.ascontiguousarray(b2[m].reshape(KC, P).T),
                "bh": bhm,
            }
        )
    return in_maps


def _unshard(results):
    outs = [np.asarray(res["out"], np.float32) for res in results]  # [130, B] each
    mu_o = np.stack([np.ascontiguousarray(o[0:OBS].T) for o in outs])
    mu_r = np.stack([np.ascontiguousarray(o[OBS : OBS + 1].T) for o in outs])
    lv_o = np.stack([np.ascontiguousarray(o[OBS + 1 : 2 * OBS + 1].T) for o in outs])
    lv_r = np.stack([np.ascontiguousarray(o[2 * OBS + 1 : NH].T) for o in outs])
    return mu_o, lv_o, mu_r, lv_r


def run(inputs, trace=False, **spmd_kwargs):
    """Run the SPMD kernel; returns ((mu_o, lv_o, mu_r, lv_r), BassKernelResults)."""
    from concourse.bass_utils import run_bass_kernel_spmd

    nc = _get_program()
    in_maps = _make_in_maps(inputs)
    res = run_bass_kernel_spmd(
        nc, in_maps, core_ids=list(range(M)), trace=trace, **spmd_kwargs
    )
    return _unshard(res.results), res


def kernel(**inputs):
    outputs, _ = run(inputs)
    return outputs


# revision 7
# speedup vs baseline: 4.9470x; 1.1360x over previous
"""Trainium2 Bass kernel for nn_EnsembleModel (ensemble MLP, M=8 models).

Sharding: one ensemble member per NeuronCore (8 models / 8 cores). Each core
runs the full batch through its model's 3-layer MLP + 4 output heads.

Layout: features on partitions, batch on the free dim ("transposed"
activations), so every layer is out[h_out, b] = W_chunk.T @ h_prev[h_in, b]
with no transposes anywhere. The input x.T and all weight reshapes are done
host-side in numpy; outputs come back as [130, B] per core and are
untransposed host-side.

Matmuls run as float32r (full fp32 storage, full-rate PE mode at free dim
>= 256). tanh + per-feature bias are fused into one ScalarE activation per
128-row chunk. The soft log-var clamp
    lv = -10 + softplus(10.5 - softplus(0.5 - lv))
is computed exactly as b - a with
    u = exp(0.5 - lv), a = ln(1 + u), b = ln(e^-10 * u + (e^-10 + e^0.5))
which needs only Exp and Ln (one ACT table-set switch per tile pair).
"""

import math

import numpy as np

M, B, OBS, ACT, H = 8, 4096, 64, 32, 1024
IN = OBS + ACT  # 96
P = 128
KC = H // P  # 8 k-chunks per 1024-dim contraction
NH = 2 * OBS + 2  # 130 head output columns: [mu_o(64), mu_r(1), v_o(64), v_r(1)]
B_T = 512
N_BT = B // B_T
MAX_LV, MIN_LV = 0.5, -10.0

_CLAMP_SCALE = float(np.exp(MIN_LV))  # e^-10
_CLAMP_BIAS = float(np.exp(MIN_LV) + np.exp(MAX_LV))  # e^-10 + e^0.5

_PROGRAM = None


def _build_program(repeat=1):
    import concourse.mybir as mybir
    from concourse import bacc
    from concourse.bass import ds, ts
    from concourse.tile import TileContext

    f32 = mybir.dt.float32
    f32r = mybir.dt.float32r
    Act = mybir.ActivationFunctionType

    nc = bacc.Bacc("TRN2", target_bir_lowering=False)

    xT = nc.dram_tensor("xT", [P, B], f32r, kind="ExternalInput")
    w0 = nc.dram_tensor("w0", [P, H], f32r, kind="ExternalInput")
    w1 = nc.dram_tensor("w1", [P, KC, H], f32r, kind="ExternalInput")
    w2 = nc.dram_tensor("w2", [P, KC, H], f32r, kind="ExternalInput")
    wh = nc.dram_tensor("wh", [P, KC, NH], f32r, kind="ExternalInput")
    b0 = nc.dram_tensor("b0", [P, KC], f32, kind="ExternalInput")
    b1 = nc.dram_tensor("b1", [P, KC], f32, kind="ExternalInput")
    b2 = nc.dram_tensor("b2", [P, KC], f32, kind="ExternalInput")
    bh = nc.dram_tensor("bh", [P, 3], f32, kind="ExternalInput")
    out = nc.dram_tensor("out", [NH, B], f32, kind="ExternalOutput")

    def r(ap):
        return ap  # tiles feeding matmuls are already float32r

    with TileContext(nc) as tc:
        with (
            tc.tile_pool(name="consts", bufs=1) as consts,
            tc.tile_pool(name="h0p", bufs=2) as h0p,
            tc.tile_pool(name="h1p", bufs=1) as h1p,
            tc.tile_pool(name="h2p", bufs=1) as h2p,
            tc.tile_pool(name="epi", bufs=2) as epi,
            tc.tile_pool(name="psum", bufs=8, space="PSUM") as psum_pool,
        ):
            xT_sb = consts.tile([P, B], f32r, tag="xT")
            w0_sb = consts.tile([P, H], f32r, tag="w0")
            w1_sb = consts.tile([P, KC, H], f32r, tag="w1")
            w2_sb = consts.tile([P, KC, H], f32r, tag="w2")
            wh_sb = consts.tile([P, KC, NH], f32r, tag="wh")
            b0_sb = consts.tile([P, KC], f32, tag="b0")
            b1_sb = consts.tile([P, KC], f32, tag="b1")
            b2_sb = consts.tile([P, KC], f32, tag="b2")
            bh_sb = consts.tile([P, 3], f32, tag="bh")
            # u = exp(0.5 - lv_pre) stash, clamped to logs in one batch at the
            # end so the ACT engine needs only one table-set switch.
            u_all = consts.tile([OBS + 1, B], f32, tag="u_all")

            # DMAs issued in first-consumption order: tile-0 input, then
            # weights in the 128-column chunks each PSUM group consumes.
            nc.sync.dma_start(xT_sb[:, ds(0, B_T)], xT[:, ds(0, B_T)])
            nc.sync.dma_start(w0_sb[:], w0[:])
            nc.sync.dma_start(b0_sb[:], b0[:])
            for c in range(KC):
                nc.sync.dma_start(w1_sb[:, :, ts(c, P)], w1[:, :, ts(c, P)])
            nc.sync.dma_start(b1_sb[:], b1[:])
            for c in range(KC):
                nc.sync.dma_start(w2_sb[:, :, ts(c, P)], w2[:, :, ts(c, P)])
            nc.sync.dma_start(b2_sb[:], b2[:])
            nc.sync.dma_start(wh_sb[:], wh[:])
            nc.sync.dma_start(bh_sb[:], bh[:])
            for j in range(1, N_BT):
                nc.sync.dma_start(xT_sb[:, ds(j * B_T, B_T)], xT[:, ds(j * B_T, B_T)])

            for j in range(N_BT * repeat):
                j = j % N_BT
                js = ds(j * B_T, B_T)

                # Layer 0: [96->128 padded, B_T] -> h0 [1024, B_T]
                h0 = h0p.tile([P, KC, B_T], f32r)
                for c in range(KC):
                    ps = psum_pool.tile([P, B_T], f32, tag="ps")
                    nc.tensor.matmul(
                        ps[:], w0_sb[:, ts(c, P)], xT_sb[:, js],
                        start=True, stop=True,
                    )
                    nc.scalar.activation(
                        h0[:, c], ps[:], Act.Tanh, bias=b0_sb[:, c : c + 1]
                    )

                # Layers 1 and 2: 1024 -> 1024, k-accumulated in PSUM
                h1 = h1p.tile([P, KC, B_T], f32r)
                for c in range(KC):
                    ps = psum_pool.tile([P, B_T], f32, tag="ps")
                    for k in range(KC):
                        nc.tensor.matmul(
                            ps[:], w1_sb[:, k, ts(c, P)], h0[:, k],
                            start=(k == 0), stop=(k == KC - 1),
                        )
                    nc.scalar.activation(
                        h1[:, c], ps[:], Act.Tanh, bias=b1_sb[:, c : c + 1]
                    )

                h2 = h2p.tile([P, KC, B_T], f32r)
                for c in range(KC):
                    ps = psum_pool.tile([P, B_T], f32, tag="ps")
                    for k in range(KC):
                        nc.tensor.matmul(
                            ps[:], w2_sb[:, k, ts(c, P)], h1[:, k],
                            start=(k == 0), stop=(k == KC - 1),
                        )
                    nc.scalar.activation(
                        h2[:, c], ps[:], Act.Tanh, bias=b2_sb[:, c : c + 1]
                    )

                # Heads: two 65-row groups ([mu_o, mu_r] and [v_o, v_r])
                ps_mu = psum_pool.tile([P, B_T], f32, tag="ps")
                ps_lv = psum_pool.tile([P, B_T], f32, tag="ps")
                for k in range(KC):
                    nc.tensor.matmul(
                        ps_mu[0 : OBS + 1], wh_sb[:, k, 0 : OBS + 1], h2[:, k],
                        start=(k == 0), stop=(k == KC - 1),
                    )
                for k in range(KC):
                    nc.tensor.matmul(
                        ps_lv[0 : OBS + 1], wh_sb[:, k, OBS + 1 : NH], h2[:, k],
                        start=(k == 0), stop=(k == KC - 1),
                    )

                # mu bias-add on the (otherwise idle) vector engine
                mu_sb = epi.tile([OBS + 1, B_T], f32, tag="mu")
                nc.vector.tensor_scalar_add(
                    mu_sb[:], ps_mu[0 : OBS + 1], bh_sb[0 : OBS + 1, 0:1]
                )
                nc.sync.dma_start(out[0 : OBS + 1, js], mu_sb[:])

                # u = exp(0.5 - (pre + bias_v)); Exp is in the same ACT table
                # set as Tanh, so this adds no switch.
                nc.scalar.activation(
                    u_all[:, js], ps_lv[0 : OBS + 1], Act.Exp,
                    bias=bh_sb[0 : OBS + 1, 1:2], scale=-1.0,
                )

            # Scheduler fence: keep the Ln block after ALL per-tile ACT work
            # so the ACT table set switches exactly once.
            tc.no_sync_barrier()

            # Batched clamp: lv = ln(e^-10*u + (e^-10 + e^0.5)) - ln(1 + u).
            # One table-set switch for the whole kernel.
            for j in range(N_BT):
                js = ds(j * B_T, B_T)
                a_sb = epi.tile([OBS + 1, B_T], f32, tag="a")
                nc.scalar.activation(a_sb[:], u_all[:, js], Act.Ln, bias=1.0)
                b_sb = epi.tile([OBS + 1, B_T], f32, tag="b")
                nc.scalar.activation(
                    b_sb[:], u_all[:, js], Act.Ln,
                    bias=bh_sb[0 : OBS + 1, 2:3], scale=_CLAMP_SCALE,
                )
                nc.vector.tensor_sub(b_sb[:], b_sb[:], a_sb[:])
                nc.sync.dma_start(out[OBS + 1 : NH, js], b_sb[:])

    nc.finalize()
    return nc


def _get_program():
    global _PROGRAM
    if _PROGRAM is None:
        _PROGRAM = _build_program()
    return _PROGRAM


def _get_repeat_program(repeat):
    return _build_program(repeat=repeat)


def _make_in_maps(inputs):
    obs = np.asarray(inputs["observation"], np.float32)
    act = np.asarray(inputs["action"], np.float32)
    x = np.concatenate([obs, act], axis=1)  # [B, IN]
    xT = np.zeros((P, B), np.float32)
    xT[:IN] = x.T

    W0, b0 = np.asarray(inputs["W0"], np.float32), np.asarray(inputs["b0"], np.float32)
    W1, b1 = np.asarray(inputs["W1"], np.float32), np.asarray(inputs["b1"], np.float32)
    W2, b2 = np.asarray(inputs["W2"], np.float32), np.asarray(inputs["b2"], np.float32)
    Wmu_o, bmu_o = np.asarray(inputs["Wmu_o"], np.float32), np.asarray(inputs["bmu_o"], np.float32)
    Wmu_r, bmu_r = np.asarray(inputs["Wmu_r"], np.float32), np.asarray(inputs["bmu_r"], np.float32)
    Wv_o, bv_o = np.asarray(inputs["Wv_o"], np.float32), np.asarray(inputs["bv_o"], np.float32)
    Wv_r, bv_r = np.asarray(inputs["Wv_r"], np.float32), np.asarray(inputs["bv_r"], np.float32)

    def kchunk(w, ncols):
        # [H, ncols] -> [128, KC, ncols] with row index = ko*128 + ki
        return np.ascontiguousarray(w.reshape(KC, P, ncols).transpose(1, 0, 2))

    in_maps = []
    for m in range(M):
        w0p = np.zeros((P, H), np.float32)
        w0p[:IN] = W0[m]
        whm = np.concatenate([Wmu_o[m], Wmu_r[m], Wv_o[m], Wv_r[m]], axis=1)  # [H, NH]
        bhm = np.zeros((P, 3), np.float32)
        bhm[0:OBS, 0] = bmu_o[m]
        bhm[OBS, 0] = bmu_r[m, 0]
        bhm[0:OBS, 1] = MAX_LV - bv_o[m]
        bhm[OBS, 1] = MAX_LV - bv_r[m, 0]
        bhm[:, 2] = _CLAMP_BIAS
        in_maps.append(
            {
                "xT": xT,
                "w0": w0p,
                "w1": kchunk(W1[m], H),
                "w2": kchunk(W2[m], H),
                "wh": kchunk(whm, NH),
                "b0": np.ascontiguousarray(b0[m].reshape(KC, P).T),
                "b1": np.ascontiguousarray(b1[m].reshape(KC, P).T),
                "b2": np.ascontiguousarray(b2[m].reshape(KC, P).T),
                "bh": bhm,
            }
        )
    return in_maps


def _unshard(results):
    outs = [np.asarray(res["out"], np.float32) for res in results]  # [130, B] each
    mu_o = np.stack([np.ascontiguousarray(o[0:OBS].T) for o in outs])
    mu_r = np.stack([np.ascontiguousarray(o[OBS : OBS + 1].T) for o in outs])
    lv_o = np.stack([np.ascontiguousarray(o[OBS + 1 : 2 * OBS + 1].T) for o in outs])
    lv_r = np.stack([np.ascontiguousarray(o[2 * OBS + 1 : NH].T) for o in outs])
    return mu_o, lv_o, mu_r, lv_r


def run(inputs, trace=False, **spmd_kwargs):
    """Run the SPMD kernel; returns ((mu_o, lv_o, mu_r, lv_r), BassKernelResults)."""
    from concourse.bass_utils import run_bass_kernel_spmd

    nc = _get_program()
    in_maps = _make_in_maps(inputs)
    res = run_bass_kernel_spmd(
        nc, in_maps, core_ids=list(range(M)), trace=trace, **spmd_kwargs
    )
    return _unshard(res.results), res


def kernel(**inputs):
    outputs, _ = run(inputs)
    return outputs


# revision 8
# speedup vs baseline: 4.9746x; 1.0056x over previous
"""Trainium2 Bass kernel for nn_EnsembleModel (ensemble MLP, M=8 models).

Sharding: one ensemble member per NeuronCore (8 models / 8 cores). Each core
runs the full batch through its model's 3-layer MLP + 4 output heads.

Layout: features on partitions, batch on the free dim ("transposed"
activations), so every layer is out[h_out, b] = W_chunk.T @ h_prev[h_in, b]
with no transposes anywhere. The input x.T and all weight reshapes are done
host-side in numpy; outputs come back as [130, B] per core and are
untransposed host-side.

Matmuls run as float32r (full fp32 storage, full-rate PE mode at free dim
>= 256). tanh + per-feature bias are fused into one ScalarE activation per
128-row chunk. The soft log-var clamp
    lv = -10 + softplus(10.5 - softplus(0.5 - lv))
is computed exactly as b - a with
    u = exp(0.5 - lv), a = ln(1 + u), b = ln(e^-10 * u + (e^-10 + e^0.5))
which needs only Exp and Ln (one ACT table-set switch per tile pair).
"""

import math

import numpy as np

M, B, OBS, ACT, H = 8, 4096, 64, 32, 1024
IN = OBS + ACT  # 96
P = 128
KC = H // P  # 8 k-chunks per 1024-dim contraction
NH = 2 * OBS + 2  # 130 head output columns: [mu_o(64), mu_r(1), v_o(64), v_r(1)]
B_T = 512
N_BT = B // B_T
MAX_LV, MIN_LV = 0.5, -10.0

_CLAMP_SCALE = float(np.exp(MIN_LV))  # e^-10
_CLAMP_BIAS = float(np.exp(MIN_LV) + np.exp(MAX_LV))  # e^-10 + e^0.5

_PROGRAM = None


def _build_program(repeat=1):
    import concourse.mybir as mybir
    from concourse import bacc
    from concourse.bass import ds, ts
    from concourse.tile import TileContext

    f32 = mybir.dt.float32
    f32r = mybir.dt.float32r
    f16 = mybir.dt.float16
    Act = mybir.ActivationFunctionType

    nc = bacc.Bacc("TRN2", target_bir_lowering=False)

    xT = nc.dram_tensor("xT", [P, B], f16, kind="ExternalInput")
    w0 = nc.dram_tensor("w0", [P, H], f16, kind="ExternalInput")
    w1 = nc.dram_tensor("w1", [P, KC, H], f16, kind="ExternalInput")
    w2 = nc.dram_tensor("w2", [P, KC, H], f16, kind="ExternalInput")
    wh = nc.dram_tensor("wh", [P, KC, NH], f16, kind="ExternalInput")
    b0 = nc.dram_tensor("b0", [P, KC], f32, kind="ExternalInput")
    b1 = nc.dram_tensor("b1", [P, KC], f32, kind="ExternalInput")
    b2 = nc.dram_tensor("b2", [P, KC], f32, kind="ExternalInput")
    bh = nc.dram_tensor("bh", [P, 3], f32, kind="ExternalInput")
    out = nc.dram_tensor("out", [NH, B], f32, kind="ExternalOutput")

    def r(ap):
        return ap  # tiles feeding matmuls are already float32r

    with TileContext(nc) as tc:
        with (
            tc.tile_pool(name="consts", bufs=1) as consts,
            tc.tile_pool(name="h0p", bufs=2) as h0p,
            tc.tile_pool(name="h1p", bufs=2) as h1p,
            tc.tile_pool(name="h2p", bufs=2) as h2p,
            tc.tile_pool(name="epi", bufs=2) as epi,
            tc.tile_pool(name="psum", bufs=8, space="PSUM") as psum_pool,
        ):
            xT_sb = consts.tile([P, B], f16, tag="xT")
            w0_sb = consts.tile([P, H], f16, tag="w0")
            w1_sb = consts.tile([P, KC, H], f16, tag="w1")
            w2_sb = consts.tile([P, KC, H], f16, tag="w2")
            wh_sb = consts.tile([P, KC, NH], f16, tag="wh")
            b0_sb = consts.tile([P, KC], f32, tag="b0")
            b1_sb = consts.tile([P, KC], f32, tag="b1")
            b2_sb = consts.tile([P, KC], f32, tag="b2")
            bh_sb = consts.tile([P, 3], f32, tag="bh")
            # u = exp(0.5 - lv_pre) stash, clamped to logs in one batch at the
            # end so the ACT engine needs only one table-set switch.
            u_all = consts.tile([OBS + 1, B], f32, tag="u_all")

            # DMAs issued in first-consumption order: tile-0 input, then
            # weights in the 128-column chunks each PSUM group consumes.
            nc.sync.dma_start(xT_sb[:, ds(0, B_T)], xT[:, ds(0, B_T)])
            nc.sync.dma_start(w0_sb[:], w0[:])
            nc.sync.dma_start(b0_sb[:], b0[:])
            for c in range(KC):
                nc.sync.dma_start(w1_sb[:, :, ts(c, P)], w1[:, :, ts(c, P)])
            nc.sync.dma_start(b1_sb[:], b1[:])
            for c in range(KC):
                nc.sync.dma_start(w2_sb[:, :, ts(c, P)], w2[:, :, ts(c, P)])
            nc.sync.dma_start(b2_sb[:], b2[:])
            nc.sync.dma_start(wh_sb[:], wh[:])
            nc.sync.dma_start(bh_sb[:], bh[:])
            for j in range(1, N_BT):
                nc.sync.dma_start(xT_sb[:, ds(j * B_T, B_T)], xT[:, ds(j * B_T, B_T)])

            for j in range(N_BT * repeat):
                j = j % N_BT
                js = ds(j * B_T, B_T)

                # Layer 0: [96->128 padded, B_T] -> h0 [1024, B_T]
                h0 = h0p.tile([P, KC, B_T], f16)
                for c in range(KC):
                    ps = psum_pool.tile([P, B_T], f32, tag="ps")
                    nc.tensor.matmul(
                        ps[:], w0_sb[:, ts(c, P)], xT_sb[:, js],
                        start=True, stop=True,
                    )
                    nc.scalar.activation(
                        h0[:, c], ps[:], Act.Tanh, bias=b0_sb[:, c : c + 1]
                    )

                # Layers 1 and 2: 1024 -> 1024, k-accumulated in PSUM
                h1 = h1p.tile([P, KC, B_T], f16)
                for c in range(KC):
                    ps = psum_pool.tile([P, B_T], f32, tag="ps")
                    for k in range(KC):
                        nc.tensor.matmul(
                            ps[:], w1_sb[:, k, ts(c, P)], h0[:, k],
                            start=(k == 0), stop=(k == KC - 1),
                        )
                    nc.scalar.activation(
                        h1[:, c], ps[:], Act.Tanh, bias=b1_sb[:, c : c + 1]
                    )

                h2 = h2p.tile([P, KC, B_T], f16)
                for c in range(KC):
                    ps = psum_pool.tile([P, B_T], f32, tag="ps")
                    for k in range(KC):
                        nc.tensor.matmul(
                            ps[:], w2_sb[:, k, ts(c, P)], h1[:, k],
                            start=(k == 0), stop=(k == KC - 1),
                        )
                    nc.scalar.activation(
                        h2[:, c], ps[:], Act.Tanh, bias=b2_sb[:, c : c + 1]
                    )

                # Heads: two 65-row groups ([mu_o, mu_r] and [v_o, v_r])
                ps_mu = psum_pool.tile([P, B_T], f32, tag="ps")
                ps_lv = psum_pool.tile([P, B_T], f32, tag="ps")
                for k in range(KC):
                    nc.tensor.matmul(
                        ps_mu[0 : OBS + 1], wh_sb[:, k, 0 : OBS + 1], h2[:, k],
                        start=(k == 0), stop=(k == KC - 1),
                    )
                for k in range(KC):
                    nc.tensor.matmul(
                        ps_lv[0 : OBS + 1], wh_sb[:, k, OBS + 1 : NH], h2[:, k],
                        start=(k == 0), stop=(k == KC - 1),
                    )

                # mu bias-add on the (otherwise idle) vector engine
                mu_sb = epi.tile([OBS + 1, B_T], f32, tag="mu")
                nc.vector.tensor_scalar_add(
                    mu_sb[:], ps_mu[0 : OBS + 1], bh_sb[0 : OBS + 1, 0:1]
                )
                nc.sync.dma_start(out[0 : OBS + 1, js], mu_sb[:])

                # u = exp(0.5 - (pre + bias_v)); Exp is in the same ACT table
                # set as Tanh, so this adds no switch.
                nc.scalar.activation(
                    u_all[:, js], ps_lv[0 : OBS + 1], Act.Exp,
                    bias=bh_sb[0 : OBS + 1, 1:2], scale=-1.0,
                )

            # Scheduler fence: keep the Ln block after ALL per-tile ACT work
            # so the ACT table set switches exactly once.
            tc.no_sync_barrier()

            # Batched clamp: lv = ln(e^-10*u + (e^-10 + e^0.5)) - ln(1 + u).
            # One table-set switch for the whole kernel.
            for j in range(N_BT):
                js = ds(j * B_T, B_T)
                a_sb = epi.tile([OBS + 1, B_T], f32, tag="a")
                nc.scalar.activation(a_sb[:], u_all[:, js], Act.Ln, bias=1.0)
                b_sb = epi.tile([OBS + 1, B_T], f32, tag="b")
                nc.scalar.activation(
                    b_sb[:], u_all[:, js], Act.Ln,
                    bias=bh_sb[0 : OBS + 1, 2:3], scale=_CLAMP_SCALE,
                )
                nc.vector.tensor_sub(b_sb[:], b_sb[:], a_sb[:])
                nc.sync.dma_start(out[OBS + 1 : NH, js], b_sb[:])

    nc.finalize()
    return nc


def _get_program():
    global _PROGRAM
    if _PROGRAM is None:
        _PROGRAM = _build_program()
    return _PROGRAM


def _get_repeat_program(repeat):
    return _build_program(repeat=repeat)


def _make_in_maps(inputs):
    obs = np.asarray(inputs["observation"], np.float32)
    act = np.asarray(inputs["action"], np.float32)
    x = np.concatenate([obs, act], axis=1)  # [B, IN]
    xT = np.zeros((P, B), np.float16)
    xT[:IN] = x.T.astype(np.float16)

    W0, b0 = np.asarray(inputs["W0"], np.float32), np.asarray(inputs["b0"], np.float32)
    W1, b1 = np.asarray(inputs["W1"], np.float32), np.asarray(inputs["b1"], np.float32)
    W2, b2 = np.asarray(inputs["W2"], np.float32), np.asarray(inputs["b2"], np.float32)
    Wmu_o, bmu_o = np.asarray(inputs["Wmu_o"], np.float32), np.asarray(inputs["bmu_o"], np.float32)
    Wmu_r, bmu_r = np.asarray(inputs["Wmu_r"], np.float32), np.asarray(inputs["bmu_r"], np.float32)
    Wv_o, bv_o = np.asarray(inputs["Wv_o"], np.float32), np.asarray(inputs["bv_o"], np.float32)
    Wv_r, bv_r = np.asarray(inputs["Wv_r"], np.float32), np.asarray(inputs["bv_r"], np.float32)

    def kchunk(w, ncols):
        # [H, ncols] -> [128, KC, ncols] with row index = ko*128 + ki
        return np.ascontiguousarray(
            w.reshape(KC, P, ncols).transpose(1, 0, 2).astype(np.float16)
        )

    in_maps = []
    for m in range(M):
        w0p = np.zeros((P, H), np.float16)
        w0p[:IN] = W0[m].astype(np.float16)
        whm = np.concatenate([Wmu_o[m], Wmu_r[m], Wv_o[m], Wv_r[m]], axis=1)  # [H, NH]
        bhm = np.zeros((P, 3), np.float32)
        bhm[0:OBS, 0] = bmu_o[m]
        bhm[OBS, 0] = bmu_r[m, 0]
        bhm[0:OBS, 1] = MAX_LV - bv_o[m]
        bhm[OBS, 1] = MAX_LV - bv_r[m, 0]
        bhm[:, 2] = _CLAMP_BIAS
        in_maps.append(
            {
                "xT": xT,
                "w0": w0p,
                "w1": kchunk(W1[m], H),
                "w2": kchunk(W2[m], H),
                "wh": kchunk(whm, NH),
                "b0": np.ascontiguousarray(b0[m].reshape(KC, P).T),
                "b1": np.ascontiguousarray(b1[m].reshape(KC, P).T),
                "b2": np.ascontiguousarray(b2[m].reshape(KC, P).T),
                "bh": bhm,
            }
        )
    return in_maps


def _unshard(results):
    outs = [np.asarray(res["out"], np.float32) for res in results]  # [130, B] each
    mu_o = np.stack([np.ascontiguousarray(o[0:OBS].T) for o in outs])
    mu_r = np.stack([np.ascontiguousarray(o[OBS : OBS + 1].T) for o in outs])
    lv_o = np.stack([np.ascontiguousarray(o[OBS + 1 : 2 * OBS + 1].T) for o in outs])
    lv_r = np.stack([np.ascontiguousarray(o[2 * OBS + 1 : NH].T) for o in outs])
    return mu_o, lv_o, mu_r, lv_r


def run(inputs, trace=False, **spmd_kwargs):
    """Run the SPMD kernel; returns ((mu_o, lv_o, mu_r, lv_r), BassKernelResults)."""
    from concourse.bass_utils import run_bass_kernel_spmd

    nc = _get_program()
    in_maps = _make_in_maps(inputs)
    res = run_bass_kernel_spmd(
        nc, in_maps, core_ids=list(range(M)), trace=trace, **spmd_kwargs
    )
    return _unshard(res.results), res


def kernel(**inputs):
    outputs, _ = run(inputs)
    return outputs
